# revision 1
# baseline (speedup 1.0000x reference)
"""DeepSeek-V2 MoE grouped-GEMM expert FFN (SwiGLU) on 8 Trainium2 NeuronCores.

Expert-parallel: tokens are pre-sorted by expert; each core gets a set of
(expert weights, <=512-token tile) work items. All three GEMMs keep the
weights as the stationary (lhsT) operand and stream activations token-major:

  gate^T[n,tok] = sum_k  gate_w[k,n]^T @ x^T[k,tok]     (k over HIDDEN/128)
  act  = silu(gate^T) * up^T        (bf16)
  y^T[h,tok]   = sum_f  down_w[f,h]^T @ act[f,tok]      (f over INTER/128)

Weights are host-rearranged per (tile, out-block) into [128, nk*128] slabs so
every weight DMA is a single large linear transfer and the device consumes
weights in exactly streaming order (each weight element is used once).
gate+up slabs are packed into one tensor (one DMA per n), x and down slabs are
loaded in grouped DMAs -- HWDGE descriptor generation costs ~600ns per
dma_start, so fewer/larger DMAs keep the ramp issue-bound time low.
Compute dtype bf16, accumulation fp32 in PSUM, output fp32.
"""

import sys

if "/opt/trn_rl_repo" not in sys.path:
    sys.path.insert(0, "/opt/trn_rl_repo")

import numpy as np
import ml_dtypes

N_CORES = 8
HIDDEN = 2048
INTER = 1408
TOK_TILE = 512
KT = HIDDEN // 128  # 16
FT = INTER // 128   # 11

_NC_CACHE = {}


def _build_nc(T):
    """Bass program for one core: T independent (weights, 512-token) work items."""
    import concourse.bacc as bacc
    import concourse.mybir as mybir
    import concourse.tile as tile

    bf16 = mybir.dt.bfloat16
    f32 = mybir.dt.float32

    QG = 4 if KT % 4 == 0 else 1   # x tiles per grouped DMA
    NQ = KT // QG
    PG = 2 if KT % 2 == 0 else 1   # down-proj slabs per grouped DMA
    NP = KT // PG

    nc = bacc.Bacc("TRN2", target_bir_lowering=False, debug=False)
    xt = nc.dram_tensor("xt", [T, KT, 128, TOK_TILE], bf16, kind="ExternalInput")
    guw = nc.dram_tensor("guw", [T, FT, 128, 2 * HIDDEN], bf16, kind="ExternalInput")
    dw = nc.dram_tensor("dw", [T, KT, 128, INTER], bf16, kind="ExternalInput")
    yt = nc.dram_tensor("yt", [T, KT, 128, TOK_TILE], f32, kind="ExternalOutput")

    with tile.TileContext(nc) as tc:
        with (
            tc.tile_pool(name="xpool", bufs=2 * NQ) as xpool,
            tc.tile_pool(name="wpool", bufs=5) as wpool,
            tc.tile_pool(name="apool", bufs=2 * FT) as apool,
            tc.tile_pool(name="spool", bufs=3) as spool,
            tc.tile_pool(name="opool", bufs=4) as opool,
            tc.tile_pool(name="psA", bufs=2, space="PSUM") as psA,
            tc.tile_pool(name="psB", bufs=3, space="PSUM") as psB,
        ):
            for t in range(T):
                # loads ride sync's HWDGE ring in exact consume order
                guw0 = wpool.tile([128, 2 * HIDDEN], bf16, name=f"guw_{t}_0", tag="guw")
                xqs = [
                    xpool.tile([128, QG, TOK_TILE], bf16, name=f"x_{t}_{q}", tag="x")
                    for q in range(NQ)
                ]

                def ld_xq(q, t=t, xqs=xqs):
                    src = xt[t, q * QG:(q + 1) * QG, :, :].rearrange("k r c -> r k c")
                    nc.sync.dma_start(xqs[q][:], src)

                if t == 0 and QG == 4:
                    # interleave n=0 weight chunks with x quads in consume order
                    H2 = HIDDEN // 2
                    nc.sync.dma_start(guw0[:, 0:H2], guw[t, 0, :, 0:H2])
                    ld_xq(0)
                    ld_xq(1)
                    nc.sync.dma_start(guw0[:, H2:HIDDEN], guw[t, 0, :, H2:HIDDEN])
                    ld_xq(2)
                    ld_xq(3)
                    nc.sync.dma_start(guw0[:, HIDDEN:], guw[t, 0, :, HIDDEN:])
                else:
                    nc.sync.dma_start(guw0[:], guw[t, 0, :, :])
                    for q in range(NQ):
                        ld_xq(q)

                def xk(k, xqs=xqs):
                    return xqs[k // QG][:, k % QG, :]

                acts = []
                for n in range(FT):
                    if n == 0:
                        guwt = guw0
                    else:
                        guwt = wpool.tile([128, 2 * HIDDEN], bf16,
                                          name=f"guw_{t}_{n}", tag="guw")
                        nc.sync.dma_start(guwt[:], guw[t, n, :, :])

                    psg = psA.tile([128, TOK_TILE], f32, name=f"psg_{t}_{n}", tag="psg")
                    psu = psA.tile([128, TOK_TILE], f32, name=f"psu_{t}_{n}", tag="psu")
                    for k in range(KT):
                        nc.tensor.matmul(
                            psg[:], guwt[:, k * 128:(k + 1) * 128], xk(k),
                            start=(k == 0), stop=(k == KT - 1),
                        )
                    for k in range(KT):
                        nc.tensor.matmul(
                            psu[:], guwt[:, HIDDEN + k * 128:HIDDEN + (k + 1) * 128],
                            xk(k), start=(k == 0), stop=(k == KT - 1),
                        )

                    sg = spool.tile([128, TOK_TILE], f32, name=f"sg_{t}_{n}", tag="sg")
                    nc.scalar.activation(
                        sg[:], psg[:], mybir.ActivationFunctionType.Silu
                    )
                    at = apool.tile([128, TOK_TILE], bf16, name=f"act_{t}_{n}", tag="act")
                    nc.vector.tensor_mul(at[:], sg[:], psu[:])
                    acts.append(at)

                for p in range(NP):
                    dwt = wpool.tile([128, PG, INTER], bf16, name=f"dw_{t}_{p}",
                                     tag="dw", bufs=3)
                    src = dw[t, p * PG:(p + 1) * PG, :, :].rearrange("h r c -> r h c")
                    nc.sync.dma_start(dwt[:], src)
                    for j in range(PG):
                        h = p * PG + j
                        psy = psB.tile([128, TOK_TILE], f32, name=f"psy_{t}_{h}", tag="psy")
                        for f in range(FT):
                            nc.tensor.matmul(
                                psy[:], dwt[:, j, f * 128:(f + 1) * 128], acts[f][:],
                                start=(f == 0), stop=(f == FT - 1),
                            )
                        ot = opool.tile([128, TOK_TILE], f32, name=f"o_{t}_{h}", tag="o")
                        # stores ride the ACT engine's HWDGE ring so they never
                        # head-of-line block the load stream; the very last store
                        # is split so copy/DMA/receipt overlap after the final MM.
                        if t == T - 1 and h == KT - 1:
                            half = TOK_TILE // 2
                            for ci in range(2):
                                sl = slice(ci * half, (ci + 1) * half)
                                nc.vector.tensor_copy(ot[:, sl], psy[:, sl])
                                nc.scalar.dma_start(yt[t, h, :, sl], ot[:, sl])
                        else:
                            nc.vector.tensor_copy(ot[:], psy[:])
                            nc.scalar.dma_start(yt[t, h, :, :], ot[:])

    nc.compile()
    return nc


def _get_nc(T):
    if T not in _NC_CACHE:
        _NC_CACHE[T] = _build_nc(T)
    return _NC_CACHE[T]


def kernel(hidden_states, gate_w, up_w, down_w, group_sizes):
    from concourse.bass_utils import run_bass_kernel_spmd

    bf16 = ml_dtypes.bfloat16
    X = np.ascontiguousarray(np.asarray(hidden_states))
    gs = np.asarray(group_sizes).astype(np.int64)
    num_tokens, H = X.shape
    E, _, F = gate_w.shape
    assert H == HIDDEN and F == INTER

    # work-item list: (expert, row_start, nrows), rows grouped by expert
    tiles = []
    off = 0
    for e in range(E):
        m = int(gs[e])
        s = 0
        while s < m:
            nr = min(TOK_TILE, m - s)
            tiles.append((e, off + s, nr))
            s += nr
        off += m

    out = np.zeros((num_tokens, H), dtype=np.float32)
    if not tiles:
        return out
    while len(tiles) % N_CORES:
        tiles.append((tiles[0][0], 0, 0))  # dummy pad tile; output discarded
    T = len(tiles) // N_CORES

    Xb = X.astype(bf16)
    Gb = np.asarray(gate_w).astype(bf16)
    Ub = np.asarray(up_w).astype(bf16)
    Db = np.asarray(down_w).astype(bf16)

    # per-expert weight rearrangement (cached per expert within this call)
    gu_cache, d_cache = {}, {}

    def gu_r(e):
        if e not in gu_cache:
            g = Gb[e].reshape(KT, 128, FT, 128).transpose(2, 1, 0, 3).reshape(
                FT, 128, HIDDEN)
            u = Ub[e].reshape(KT, 128, FT, 128).transpose(2, 1, 0, 3).reshape(
                FT, 128, HIDDEN)
            gu_cache[e] = np.concatenate([g, u], axis=-1)
        return gu_cache[e]

    def d_r(e):
        if e not in d_cache:
            d_cache[e] = np.ascontiguousarray(
                Db[e].reshape(FT, 128, KT, 128).transpose(2, 1, 0, 3)
            ).reshape(KT, 128, INTER)
        return d_cache[e]

    in_maps = []
    for c in range(N_CORES):
        tl = tiles[c * T:(c + 1) * T]
        xt = np.zeros((T, KT, 128, TOK_TILE), dtype=bf16)
        guw = np.empty((T, FT, 128, 2 * HIDDEN), dtype=bf16)
        dw = np.empty((T, KT, 128, INTER), dtype=bf16)
        for i, (e, r0, nr) in enumerate(tl):
            if nr:
                xt[i, :, :, :nr] = Xb[r0:r0 + nr].T.reshape(KT, 128, nr)
            guw[i] = gu_r(e)
            dw[i] = d_r(e)
        in_maps.append({"xt": xt, "guw": guw, "dw": dw})

    nc = _get_nc(T)
    res = run_bass_kernel_spmd(nc, in_maps, core_ids=list(range(N_CORES)))

    for c in range(N_CORES):
        ytc = res.results[c]["yt"]  # [T, KT, 128, TOK_TILE] f32
        for i, (e, r0, nr) in enumerate(tiles[c * T:(c + 1) * T]):
            if nr:
                out[r0:r0 + nr] = (
                    ytc[i].transpose(2, 0, 1).reshape(TOK_TILE, H)[:nr]
                )
    return out

